# revision 8
# baseline (speedup 1.0000x reference)
"""Trainium2 8-core kernel for the folded social+UI LightGCN problem.

Strategy (all device compute in bf16, f32 PSUM accumulation):
- The two social GCNs (user_embs / user1_embs) share S -> fold into one
  [10000, 128] propagation.  Row-shard S across 8 cores; each core computes
  S_chunk @ H as PE matmuls streaming a pre-tiled bf16 S^T from HBM, with an
  AllGather of H between hops.
- The two UI GCNs share the COO graph -> fold into one [50000, 128]
  propagation.  Dest-row shard edges across cores; per hop each core
  dma_gathers source rows (bf16, 256B each) and performs the segment-sum on
  the TensorEngine as one-hot matmuls into PSUM.  One-hot matrices are built
  on the DVE from hop-invariant per-edge (local row, val) tables via
  is_equal(iota, row) * val.  AllGather of h between hops.
- int16 gather-index limit handled by splitting the padded 50176-row table
  into two 25088-row halves; edges are grouped (window, half) with free
  reordering (segment-sum is order-invariant).
- Final ego mixing, /(hop+1), and users/pos/neg batch gathers happen on the
  host during unshard.
"""
import os
import sys

sys.path.insert(0, "/opt/trn_rl_repo")

import numpy as np
import ml_dtypes

import concourse.bass as bass
import concourse.bacc as bacc
import concourse.mybir as mybir
import concourse.tile as tile
from concourse import bass_utils

bf16 = ml_dtypes.bfloat16
dt = mybir.dt

N_CORES = 8
N_USERS = 10000
N_ITEMS = 40000
N_TOT = 50000
HID = 64
HID2 = 128
HOP = 3

CH_UI = N_TOT // N_CORES          # 6250 real rows per core
PAD_UI = 6272                     # 49 * 128
NW = PAD_UI // 128                # 49 windows
TOT_UI = PAD_UI * N_CORES         # 50176
HALF_UI = TOT_UI // 2             # 25088

CH_SOC = N_USERS // N_CORES       # 1250
PAD_SOC = 1280                    # 10 * 128
TOT_SOC = PAD_SOC * N_CORES      # 10240
NM_SOC = PAD_SOC // 128           # 10 m-tiles
NK_SOC = TOT_SOC // 128           # 80 k-tiles
KG_SOC = 16                       # k-tiles per streamed slab
NKG_SOC = NK_SOC // KG_SOC        # 5 slabs

GMAX = 8                          # chunks (of 128 edges) per dma_gather (ucode ring caps ~1024 idxs)

LAST_EXEC_NS = None


# ---------------------------------------------------------------- host prep

def _pad_table(x, chunk, pad):
    """[n, f] -> [pad * N_CORES, f]; rank r rows = x[r*chunk:(r+1)*chunk] + zeros."""
    n, f = x.shape
    out = np.zeros((pad * N_CORES, f), x.dtype)
    for r in range(N_CORES):
        out[r * pad: r * pad + chunk] = x[r * chunk: (r + 1) * chunk]
    return out


def _prep_edges(A_rows, A_cols, A_vals):
    """Group edges per (core, half, window); equal chunk counts across cores.

    Returns per-core (idx_rep, rows_l, vals_l) arrays plus shared chunk
    structure chunks98 (phase-major: half A windows 0..48 then half B).
    """
    A_rows = np.asarray(A_rows).astype(np.int64)
    A_cols = np.asarray(A_cols).astype(np.int64)
    A_vals = np.asarray(A_vals).astype(np.float32)

    core = A_rows // CH_UI
    r_loc = A_rows - core * CH_UI
    w = r_loc >> 7
    wrow = r_loc & 127
    cpad = (A_cols // CH_UI) * PAD_UI + (A_cols % CH_UI)
    half = (cpad >= HALF_UI).astype(np.int64)
    idx16 = (cpad - HALF_UI * half).astype(np.int64)
    key = half * NW + w

    per_core = []
    counts = np.zeros((N_CORES, 2 * NW), np.int64)
    for k in range(N_CORES):
        m = core == k
        kk = key[m]
        order = np.argsort(kk, kind="stable")
        per_core.append((kk[order], idx16[m][order], wrow[m][order],
                         A_vals[m][order]))
        counts[k] = np.bincount(kk, minlength=2 * NW)

    chunks98 = np.maximum(1, (counts.max(axis=0) + 127) // 128)
    NCH = int(chunks98.sum())
    NI = NCH * 128
    starts = np.concatenate([[0], np.cumsum(chunks98 * 128)[:-1]])

    maps = []
    for k in range(N_CORES):
        kk, i16, wr, vv = per_core[k]
        idx_arr = np.zeros(NI, np.int64)
        row_arr = np.full(NI, 200.0, np.float32)
        val_arr = np.zeros(NI, np.float32)
        grp_start = np.concatenate([[0], np.cumsum(counts[k])[:-1]])
        for g in range(2 * NW):
            n = counts[k][g]
            s, d = grp_start[g], starts[g]
            idx_arr[d:d + n] = i16[s:s + n]
            row_arr[d:d + n] = wr[s:s + n]
            val_arr[d:d + n] = vv[s:s + n]
        idx_rep = np.tile(idx_arr.astype(np.int16).reshape(-1, 16).T, (8, 1))
        rows_l = np.ascontiguousarray(row_arr.astype(bf16).reshape(NCH, 128).T)
        vals_l = np.ascontiguousarray(val_arr.astype(bf16).reshape(NCH, 128).T)
        maps.append((idx_rep, rows_l, vals_l))
    return maps, chunks98.astype(int), NCH


def _chunk_meta(chunks98):
    """Stream-order chunk metadata: (half, window, first, last) per chunk,
    plus gather groups [(half, c0, n)] never crossing the half boundary."""
    meta = []
    for g in range(2 * NW):
        h, w = divmod(g, NW)
        n = int(chunks98[g])
        for j in range(n):
            meta.append((h, w, j == 0, j == n - 1))
    gathers = []
    c = 0
    for h in range(2):
        end = c + int(chunks98[h * NW:(h + 1) * NW].sum())
        while c < end:
            n = min(GMAX, end - c)
            gathers.append((h, c, n))
            c += n
    return meta, gathers


def _prep_social(S):
    """Per-core pre-tiled S^T: [NM_SOC, NKG_SOC, 128, KG_SOC, 128] bf16."""
    S = np.asarray(S).astype(np.float32)
    out = []
    for k in range(N_CORES):
        Sk = S[k * CH_SOC:(k + 1) * CH_SOC]            # [1250, 10000]
        Sp = np.zeros((TOT_SOC, PAD_SOC), np.float32)  # [src pad, dst local pad]
        for rs in range(N_CORES):
            Sp[rs * PAD_SOC: rs * PAD_SOC + CH_SOC, :CH_SOC] = \
                Sk[:, rs * CH_SOC:(rs + 1) * CH_SOC].T
        st = Sp.reshape(NK_SOC, 128, NM_SOC, 128).transpose(2, 0, 1, 3)
        st = st.reshape(NM_SOC, NKG_SOC, KG_SOC, 128, 128).transpose(0, 1, 3, 2, 4)
        out.append(np.ascontiguousarray(st.astype(bf16)))
    return out


# ---------------------------------------------------------------- builder

def _build(chunks98, NCH):
    meta, gathers = _chunk_meta(chunks98)
    NI = NCH * 128

    nc = bacc.Bacc(None, target_bir_lowering=False, num_swdge_queues=2)

    ego0 = nc.dram_tensor("ego0", [TOT_UI, HID2], dt.bfloat16, kind="ExternalInput")
    hsoc0 = nc.dram_tensor("hsoc0", [TOT_SOC, HID2], dt.bfloat16, kind="ExternalInput")
    ego_ch = nc.dram_tensor("ego_ch", [PAD_UI, HID2], dt.bfloat16, kind="ExternalInput")
    hsoc_ch = nc.dram_tensor("hsoc_ch", [PAD_SOC, HID2], dt.bfloat16, kind="ExternalInput")
    st_d = nc.dram_tensor("st", [NM_SOC, NKG_SOC, 128, KG_SOC, 128], dt.bfloat16,
                          kind="ExternalInput")
    idx_d = nc.dram_tensor("idx", [128, NI // 16], dt.int16, kind="ExternalInput")
    rows_d = nc.dram_tensor("rows", [128, NCH], dt.bfloat16, kind="ExternalInput")
    vals_d = nc.dram_tensor("vals", [128, NCH], dt.bfloat16, kind="ExternalInput")
    iota_d = nc.dram_tensor("iota", [128, 128], dt.bfloat16, kind="ExternalInput")

    out_ui = nc.dram_tensor("acc_ui", [PAD_UI, HID2], dt.float32, kind="ExternalOutput")
    out_soc = nc.dram_tensor("acc_soc", [PAD_SOC, HID2], dt.float32, kind="ExternalOutput")

    rg = [list(range(N_CORES))]

    with tile.TileContext(nc) as tc:
        with (
            tc.tile_pool(name="const", bufs=1) as constp,
            tc.tile_pool(name="acc", bufs=1) as accp,
            tc.tile_pool(name="hstage", bufs=1) as hstp,
            tc.tile_pool(name="msgs", bufs=2) as msgp,
            tc.tile_pool(name="onehot", bufs=2) as ohp,
            tc.tile_pool(name="slab", bufs=3) as slabp,
            tc.tile_pool(name="hsoc", bufs=1) as hsocp,
            tc.tile_pool(name="uips", bufs=4, space="PSUM") as uips,
            tc.tile_pool(name="socps", bufs=2, space="PSUM") as socps,
            tc.tile_pool(name="dram", bufs=2, space="DRAM") as dramp,
        ):
            # hop-invariant tables
            idx_sb = constp.tile([128, NI // 16], dt.int16)
            rows_sb = constp.tile([128, NCH], dt.bfloat16)
            vals_sb = constp.tile([128, NCH], dt.bfloat16)
            iota_sb = constp.tile([128, 128], dt.bfloat16)
            nc.sync.dma_start(idx_sb[:], idx_d[:])
            nc.sync.dma_start(rows_sb[:], rows_d[:])
            nc.sync.dma_start(vals_sb[:], vals_d[:])
            nc.sync.dma_start(iota_sb[:], iota_d[:])

            # persistent accumulators (f32), init = ego / h0 chunk
            acc_ui = accp.tile([128, NW, HID2], dt.float32)
            acc_soc = accp.tile([128, NM_SOC, HID2], dt.float32)
            nc.gpsimd.dma_start(
                acc_ui[:], ego_ch.rearrange("(t p) f -> p t f", p=128))
            nc.gpsimd.dma_start(
                acc_soc[:], hsoc_ch.rearrange("(t p) f -> p t f", p=128))

            ui_src = ego0          # gather source table for current hop
            soc_src = hsoc0        # social H source for current hop

            for hop in range(HOP):
                last = hop == HOP - 1

                # ---------------- social: S_chunk @ H
                hsoc_sb = hsocp.tile([128, NK_SOC, HID2], dt.bfloat16)
                nc.sync.dma_start(
                    hsoc_sb[:], soc_src.rearrange("(t p) f -> p t f", p=128))
                if not last:
                    hsoc_nx = hstp.tile([128, NM_SOC, HID2], dt.bfloat16,
                                        tag="hsoc_nx")
                for m in range(NM_SOC):
                    psum_m = socps.tile([128, HID2], dt.float32, tag="socps")
                    for kg in range(NKG_SOC):
                        slab = slabp.tile([128, KG_SOC, 128], dt.bfloat16,
                                          tag="slab")
                        nc.sync.dma_start(slab[:], st_d[m, kg])
                        for t in range(KG_SOC):
                            nc.tensor.matmul(
                                psum_m[:], slab[:, t, :],
                                hsoc_sb[:, kg * KG_SOC + t, :],
                                start=(kg == 0 and t == 0),
                                stop=(kg == NKG_SOC - 1 and t == KG_SOC - 1))
                    if not last:
                        nc.scalar.copy(hsoc_nx[:, m, :], psum_m[:])
                    nc.vector.tensor_add(acc_soc[:, m, :], acc_soc[:, m, :],
                                         psum_m[:])
                if not last:
                    soc_bounce = dramp.tile([PAD_SOC, HID2], dt.bfloat16,
                                            tag="soc_in")
                    soc_full = dramp.tile([TOT_SOC, HID2], dt.bfloat16,
                                          tag="soc_out", addr_space="Shared")
                    nc.sync.dma_start(
                        soc_bounce.rearrange("(t p) f -> p t f", p=128),
                        hsoc_nx[:])
                    nc.gpsimd.collective_compute(
                        "AllGather", mybir.AluOpType.bypass,
                        ins=[soc_bounce.opt()], outs=[soc_full.opt()],
                        replica_groups=rg)
                    soc_src = soc_full

                # ---------------- UI: gather + one-hot scatter matmuls
                if not last:
                    hf32 = hstp.tile([128, NW, HID2], dt.float32, tag="hf32")
                psum_w = {}
                for gi, (h, c0, gch) in enumerate(gathers):
                    src = ui_src[0:HALF_UI, :] if h == 0 else ui_src[HALF_UI:TOT_UI, :]
                    msgs = msgp.tile([128, gch, HID2], dt.bfloat16, tag="msgs")
                    nc.gpsimd.dma_gather(
                        out_ap=msgs[:], in_ap=src,
                        idxs_ap=idx_sb[:, c0 * 8:(c0 + gch) * 8],
                        num_idxs=gch * 128, num_idxs_reg=gch * 128,
                        elem_size=HID2, queue_num=gi % 2)
                    m_t = ohp.tile([128, gch, 128], dt.bfloat16, tag="m_t")
                    iota_b = iota_sb[:].unsqueeze(1).broadcast_to([128, gch, 128])
                    rows_b = rows_sb[:, c0:c0 + gch].unsqueeze(2) \
                        .broadcast_to([128, gch, 128])
                    vals_b = vals_sb[:, c0:c0 + gch].unsqueeze(2) \
                        .broadcast_to([128, gch, 128])
                    nc.vector.tensor_tensor(m_t[:], iota_b, rows_b,
                                            mybir.AluOpType.is_equal)
                    nc.vector.tensor_tensor(m_t[:], m_t[:], vals_b,
                                            mybir.AluOpType.mult)
                    for j in range(gch):
                        h_, w, first, lastc = meta[c0 + j]
                        if first:
                            psum_w[w] = uips.tile([128, HID2], dt.float32,
                                                  tag="uips",
                                                  name=f"uips_h{hop}_{h_}_{w}")
                        nc.tensor.matmul(psum_w[w][:], m_t[:, j, :],
                                         msgs[:, j, :], start=first, stop=lastc)
                        if lastc:
                            if last:
                                nc.vector.tensor_add(
                                    acc_ui[:, w, :], acc_ui[:, w, :],
                                    psum_w[w][:])
                            elif h_ == 0:
                                nc.scalar.copy(hf32[:, w, :], psum_w[w][:])
                            else:
                                nc.vector.tensor_add(
                                    hf32[:, w, :], hf32[:, w, :], psum_w[w][:])
                if not last:
                    nc.vector.tensor_add(acc_ui[:], acc_ui[:], hf32[:])
                    hui_nx = hstp.tile([128, NW, HID2], dt.bfloat16, tag="hui_nx")
                    nc.vector.tensor_copy(hui_nx[:], hf32[:])
                    ui_bounce = dramp.tile([PAD_UI, HID2], dt.bfloat16,
                                           tag="ui_in")
                    ui_full = dramp.tile([TOT_UI, HID2], dt.bfloat16,
                                         tag="ui_out", addr_space="Shared")
                    nc.sync.dma_start(
                        ui_bounce.rearrange("(t p) f -> p t f", p=128),
                        hui_nx[:])
                    nc.gpsimd.collective_compute(
                        "AllGather", mybir.AluOpType.bypass,
                        ins=[ui_bounce.opt()], outs=[ui_full.opt()],
                        replica_groups=rg)
                    ui_src = ui_full

            nc.sync.dma_start(out_ui.rearrange("(t p) f -> p t f", p=128),
                              acc_ui[:])
            nc.sync.dma_start(out_soc.rearrange("(t p) f -> p t f", p=128),
                              acc_soc[:])
    nc.compile()
    return nc


# ---------------------------------------------------------------- kernel

def prepare(users, pos, neg, user_embs, item_embs, user1_embs, item1_embs,
            user2_embs, item2_embs, S, A_rows, A_cols, A_vals, hop):
    """Host prep + build: returns (nc, in_maps)."""
    assert int(hop) == HOP

    user_embs = np.asarray(user_embs, np.float32)
    item_embs = np.asarray(item_embs, np.float32)
    user1_embs = np.asarray(user1_embs, np.float32)
    item1_embs = np.asarray(item1_embs, np.float32)
    user2_embs = np.asarray(user2_embs, np.float32)
    item2_embs = np.asarray(item2_embs, np.float32)

    ego = np.hstack([np.vstack([user_embs, item_embs]),
                     np.vstack([user2_embs, item2_embs])])          # [50000,128]
    hsoc = np.hstack([user_embs, user1_embs])                       # [10000,128]
    ego_pad = _pad_table(ego.astype(bf16), CH_UI, PAD_UI)           # [50176,128]
    hsoc_pad = _pad_table(hsoc.astype(bf16), CH_SOC, PAD_SOC)       # [10240,128]

    edge_maps, chunks98, NCH = _prep_edges(A_rows, A_cols, A_vals)
    st_all = _prep_social(S)
    iota_np = np.tile(np.arange(128, dtype=np.float32).astype(bf16), (128, 1))

    in_maps = []
    for k in range(N_CORES):
        idx_rep, rows_l, vals_l = edge_maps[k]
        in_maps.append({
            "ego0": ego_pad,
            "hsoc0": hsoc_pad,
            "ego_ch": np.ascontiguousarray(ego_pad[k * PAD_UI:(k + 1) * PAD_UI]),
            "hsoc_ch": np.ascontiguousarray(hsoc_pad[k * PAD_SOC:(k + 1) * PAD_SOC]),
            "st": st_all[k],
            "idx": idx_rep,
            "rows": rows_l,
            "vals": vals_l,
            "iota": iota_np,
        })

    nc = _build(chunks98, NCH)
    return nc, in_maps


def finish(results, users, pos, neg, item1_embs):
    """Assemble full outputs from per-core results."""
    inv = 1.0 / (HOP + 1)
    acc_ui = np.concatenate(
        [results[k]["acc_ui"][:CH_UI] for k in range(N_CORES)], 0) * inv
    acc_soc = np.concatenate(
        [results[k]["acc_soc"][:CH_SOC] for k in range(N_CORES)], 0) * inv

    all_user_S = acc_soc[:, :HID]
    all_user_soc1 = acc_soc[:, HID:]
    all_user_A = acc_ui[:N_USERS, :HID]
    all_item = acc_ui[N_USERS:, :HID]
    all_user_rating = acc_ui[:N_USERS, HID:]
    all_item_rating = acc_ui[N_USERS:, HID:]
    all_user = 0.5 * all_user_S + 0.5 * all_user_A

    users = np.asarray(users).astype(np.int64)
    pos = np.asarray(pos).astype(np.int64)
    neg = np.asarray(neg).astype(np.int64)
    item1_embs = np.asarray(item1_embs, np.float32)
    return (all_user[users], all_item[pos], all_item[neg],
            all_user_S, all_user_A,
            all_user_soc1[users], item1_embs[pos], item1_embs[neg],
            all_user_rating[users], all_item_rating[pos],
            all_item_rating[neg])


def kernel(users, pos, neg, user_embs, item_embs, user1_embs, item1_embs,
           user2_embs, item2_embs, S, A_rows, A_cols, A_vals, hop):
    global LAST_EXEC_NS
    nc, in_maps = prepare(users, pos, neg, user_embs, item_embs, user1_embs,
                          item1_embs, user2_embs, item2_embs, S,
                          A_rows, A_cols, A_vals, hop)
    trace = bool(int(os.environ.get("KERNEL_TRACE", "0")))
    res = bass_utils.run_bass_kernel_spmd(
        nc, in_maps, core_ids=list(range(N_CORES)), trace=trace)
    LAST_EXEC_NS = res.exec_time_ns
    return finish(res.results, users, pos, neg, item1_embs)


# revision 21
# speedup vs baseline: 1.2603x; 1.2603x over previous
"""Trainium2 8-core kernel for the folded social+UI LightGCN problem.

Strategy (all device compute in bf16, f32 PSUM accumulation):
- The two social GCNs (user_embs / user1_embs) share S -> fold into one
  [10000, 128] propagation.  Row-shard S across 8 cores; each core computes
  S_chunk @ H as PE matmuls streaming a pre-tiled bf16 S^T from HBM, with an
  AllGather of H between hops.
- The two UI GCNs share the COO graph -> fold into one [50000, 128]
  propagation.  Dest-row shard edges across cores; per hop each core
  dma_gathers source rows (bf16, 256B each, <=1024 idxs per gather) and
  performs the segment-sum on the TensorEngine as one-hot matmuls into PSUM
  (two 64-row windows packed per PSUM bank via the 64-partition col group).
  The val-scaled one-hot matrices are hop-invariant and precomputed on the
  host, streamed from HBM.  AllGather of h between hops.
- int16 gather-index limit handled by splitting the padded 50176-row table
  into two 25088-row halves; edges are grouped (window, half) with free
  reordering (segment-sum is order-invariant).
- Final ego mixing, /(hop+1), and users/pos/neg batch gathers happen on the
  host during unshard.
"""
import os
import sys

sys.path.insert(0, "/opt/trn_rl_repo")

import numpy as np
import ml_dtypes

import concourse.bass as bass
import concourse.bacc as bacc
import concourse.mybir as mybir
import concourse.tile as tile
from concourse import bass_utils

bf16 = ml_dtypes.bfloat16
dt = mybir.dt

N_CORES = 8
N_USERS = 10000
N_ITEMS = 40000
N_TOT = 50000
HID = 64
HID2 = 128
HOP = 3

CH_UI = N_TOT // N_CORES          # 6250 real rows per core
PAD_UI = 6272                     # 49 * 128
NW = PAD_UI // 128                # 49 windows
TOT_UI = PAD_UI * N_CORES         # 50176
HALF_UI = TOT_UI // 2             # 25088

CH_SOC = N_USERS // N_CORES       # 1250
PAD_SOC = 1280                    # 10 * 128
TOT_SOC = PAD_SOC * N_CORES      # 10240
NM_SOC = PAD_SOC // 128           # 10 m-tiles
NK_SOC = TOT_SOC // 128           # 80 k-tiles
KG_SOC = 16                       # k-tiles per streamed slab
NKG_SOC = NK_SOC // KG_SOC        # 5 slabs

GMAX = 8                          # chunks (of 128 edges) per dma_gather (ucode ring caps ~1024 idxs)

# split-AG layout: ui rows [0,3200) + soc m[0,5) in buf1; rest in buf2
UI1 = 3200                        # ui rows per rank in buf1 (pairs 0..24)
UI2 = PAD_UI - UI1                # 3072 (pairs 25..48)
SOC_H = PAD_SOC // 2              # 640 soc rows per buf
R1 = UI1 + SOC_H                  # 3840 rows/rank in buf1
R2 = UI2 + SOC_H                  # 3712 rows/rank in buf2
T1 = R1 * N_CORES                 # 30720
T2 = R2 * N_CORES                 # 29696

LAST_EXEC_NS = None


# ---------------------------------------------------------------- host prep

def _pad_table(x, chunk, pad):
    """[n, f] -> [pad * N_CORES, f]; rank r rows = x[r*chunk:(r+1)*chunk] + zeros."""
    n, f = x.shape
    out = np.zeros((pad * N_CORES, f), x.dtype)
    for r in range(N_CORES):
        out[r * pad: r * pad + chunk] = x[r * chunk: (r + 1) * chunk]
    return out


def _prep_edges(A_rows, A_cols, A_vals):
    """Group edges per (core, half, window); equal chunk counts across cores.

    Returns per-core (idx_rep, rows_l, vals_l) arrays plus shared chunk
    structure chunks98 (phase-major: half A windows 0..48 then half B).
    """
    A_rows = np.asarray(A_rows).astype(np.int64)
    A_cols = np.asarray(A_cols).astype(np.int64)
    A_vals = np.asarray(A_vals).astype(np.float32)

    core = A_rows // CH_UI
    r_loc = A_rows - core * CH_UI
    w = r_loc >> 7
    wrow = r_loc & 127
    cpad = (A_cols // CH_UI) * PAD_UI + (A_cols % CH_UI)
    half = (cpad >= HALF_UI).astype(np.int64)
    idx16 = (cpad - HALF_UI * half).astype(np.int64)
    key = half * NW + w

    per_core = []
    counts = np.zeros((N_CORES, 2 * NW), np.int64)
    for k in range(N_CORES):
        m = core == k
        kk = key[m]
        order = np.argsort(kk, kind="stable")
        per_core.append((kk[order], idx16[m][order], wrow[m][order],
                         A_vals[m][order]))
        counts[k] = np.bincount(kk, minlength=2 * NW)

    cmax = counts.max(axis=0)
    # guarantee >=1 chunk only for keys whose (quad, w) combination is real
    need = np.zeros(4 * NW64, np.int64)
    for ph in range(4):
        q = ph >> 1
        lo, hi = (0, UI1 // WIN) if q == 0 else (UI1 // WIN, NW64)
        need[ph * NW64 + lo: ph * NW64 + hi] = 1
    chunks98 = np.maximum(need, (cmax + 127) // 128) * need
    NCH = int(chunks98.sum())
    NI = NCH * 128
    starts = np.concatenate([[0], np.cumsum(chunks98 * 128)[:-1]])

    maps = []
    for k in range(N_CORES):
        kk, i16, wr, vv = per_core[k]
        idx_arr = np.zeros(NI, np.int64)
        row_arr = np.full(NI, 200.0, np.float32)
        val_arr = np.zeros(NI, np.float32)
        grp_start = np.concatenate([[0], np.cumsum(counts[k])[:-1]])
        for g in range(2 * NW):
            n = counts[k][g]
            s, d = grp_start[g], starts[g]
            idx_arr[d:d + n] = i16[s:s + n]
            row_arr[d:d + n] = wr[s:s + n]
            val_arr[d:d + n] = vv[s:s + n]
        idx_rep = np.tile(idx_arr.astype(np.int16).reshape(-1, 16).T, (8, 1))
        rows_l = np.ascontiguousarray(row_arr.astype(bf16).reshape(NCH, 128).T)
        vals_l = np.ascontiguousarray(val_arr.astype(bf16).reshape(NCH, 128).T)
        maps.append((idx_rep, rows_l, vals_l))
    return maps, chunks98.astype(int), NCH


def _chunk_meta(chunks98):
    """Stream-order chunk metadata: (half, window, first, last) per chunk,
    plus gather groups [(half, c0, n)] never crossing the half boundary."""
    meta = []
    for g in range(2 * NW):
        h, w = divmod(g, NW)
        n = int(chunks98[g])
        for j in range(n):
            meta.append((h, w, j == 0, j == n - 1))
    gathers = []
    c = 0
    for h in range(2):
        end = c + int(chunks98[h * NW:(h + 1) * NW].sum())
        while c < end:
            n = min(GMAX, end - c)
            gathers.append((h, c, n))
            c += n
    return meta, gathers


def _prep_social(S):
    """Per-core pre-tiled S^T: [NM_SOC, NKG_SOC, 128, KG_SOC, 128] bf16."""
    S = np.asarray(S).astype(np.float32)
    out = []
    for k in range(N_CORES):
        Sk = S[k * CH_SOC:(k + 1) * CH_SOC]            # [1250, 10000]
        Sp = np.zeros((TOT_SOC, PAD_SOC), np.float32)  # [src pad, dst local pad]
        for rs in range(N_CORES):
            Sp[rs * PAD_SOC: rs * PAD_SOC + CH_SOC, :CH_SOC] = \
                Sk[:, rs * CH_SOC:(rs + 1) * CH_SOC].T
        st = Sp.reshape(NK_SOC, 128, NM_SOC, 128).transpose(2, 0, 1, 3)
        st = st.reshape(NM_SOC, NKG_SOC, KG_SOC, 128, 128).transpose(0, 1, 3, 2, 4)
        out.append(np.ascontiguousarray(st.astype(bf16)))
    return out


# ---------------------------------------------------------------- builder

def _build(chunks98, NCH):
    meta, gathers = _chunk_meta(chunks98)
    NI = NCH * 128

    nc = bacc.Bacc(None, target_bir_lowering=False, num_swdge_queues=2)

    ego0_a = nc.dram_tensor("ego0_a", [T1, HID2], dt.bfloat16, kind="ExternalInput")
    ego0_b = nc.dram_tensor("ego0_b", [T2, HID2], dt.bfloat16, kind="ExternalInput")
    ego_ch = nc.dram_tensor("ego_ch", [PAD_UI, HID2], dt.bfloat16, kind="ExternalInput")
    hsoc_ch = nc.dram_tensor("hsoc_ch", [PAD_SOC, HID2], dt.bfloat16, kind="ExternalInput")
    st_d = nc.dram_tensor("st", [NM_SOC, NKG_SOC, 128, KG_SOC, 128], dt.bfloat16,
                          kind="ExternalInput")
    idx_d = nc.dram_tensor("idx", [128, NI // 16], dt.int16, kind="ExternalInput")
    m_d = nc.dram_tensor("m", [128, NCH, 128], dt.bfloat16, kind="ExternalInput")

    out_ui = nc.dram_tensor("acc_ui", [PAD_UI, HID2], dt.float32, kind="ExternalOutput")
    out_soc = nc.dram_tensor("acc_soc", [PAD_SOC, HID2], dt.float32, kind="ExternalOutput")

    rg = [list(range(N_CORES))]

    with tile.TileContext(nc) as tc:
        with (
            tc.tile_pool(name="const", bufs=1) as constp,
            tc.tile_pool(name="acc", bufs=1) as accp,
            tc.tile_pool(name="hstage", bufs=1) as hstp,
            tc.tile_pool(name="msgs", bufs=10) as msgp,
            tc.tile_pool(name="onehot", bufs=10) as ohp,
            tc.tile_pool(name="slab", bufs=4) as slabp,
            tc.tile_pool(name="hsoc", bufs=1) as hsocp,
            tc.tile_pool(name="uips", bufs=6, space="PSUM") as uips,
            tc.tile_pool(name="socps", bufs=2, space="PSUM") as socps,
            tc.tile_pool(name="dram", bufs=2, space="DRAM") as dramp,
        ):
            # hop-invariant tables
            idx_sb = constp.tile([128, NI // 16], dt.int16)
            nc.sync.dma_start(idx_sb[:], idx_d[:])

            # persistent accumulators (f32), init = ego / h0 chunk
            acc_ui = accp.tile([128, NW, HID2], dt.float32)
            acc_soc = accp.tile([128, NM_SOC, HID2], dt.float32)
            nc.gpsimd.dma_start(
                acc_ui[:], ego_ch.rearrange("(t p) f -> p t f", p=128))
            nc.gpsimd.dma_start(
                acc_soc[:], hsoc_ch.rearrange("(t p) f -> p t f", p=128))

            src1, src2 = ego0_a, ego0_b   # AG buffers of the previous hop

            for hop in range(HOP):
                last = hop == HOP - 1

                # ---------------- social H for this hop (from prev AG buffers)
                hsoc_sb = hsocp.tile([128, NK_SOC, HID2], dt.bfloat16)
                for r in range(N_CORES):
                    nc.sync.dma_start(
                        hsoc_sb[:, r * NM_SOC: r * NM_SOC + 5, :],
                        src1[r * R1 + UI1:(r + 1) * R1, :]
                        .rearrange("(t p) f -> p t f", p=128))
                    nc.sync.dma_start(
                        hsoc_sb[:, r * NM_SOC + 5:(r + 1) * NM_SOC, :],
                        src2[r * R2 + UI2:(r + 1) * R2, :]
                        .rearrange("(t p) f -> p t f", p=128))
                if not last:
                    hf32 = hstp.tile([128, NW, HID2], dt.float32, tag="hf32")
                    hui_nx = hstp.tile([128, NW, HID2], dt.bfloat16,
                                       tag="hui_nx")
                    hsoc_nx = hstp.tile([128, NM_SOC, HID2], dt.bfloat16,
                                        tag="hsoc_nx")
                psum_w = {}
                if not last:
                    nbuf1 = dramp.tile([T1, HID2], dt.bfloat16, tag="nbuf1",
                                       addr_space="Shared",
                                       name=f"nbuf1_{hop}")
                    nbuf2 = dramp.tile([T2, HID2], dt.bfloat16, tag="nbuf2",
                                       addr_space="Shared",
                                       name=f"nbuf2_{hop}")

                def social_half(mlo, mhi):
                    for m in range(mlo, mhi):
                        psum_m = socps.tile([128, HID2], dt.float32,
                                            tag="socps", name=f"soc{hop}_{m}")
                        for kg in range(NKG_SOC):
                            slab = slabp.tile([128, KG_SOC, 128], dt.bfloat16,
                                              tag="slab", name=f"sl{hop}_{m}_{kg}")
                            nc.sync.dma_start(slab[:], st_d[m, kg])
                            for t in range(KG_SOC):
                                nc.tensor.matmul(
                                    psum_m[:], slab[:, t, :],
                                    hsoc_sb[:, kg * KG_SOC + t, :],
                                    start=(kg == 0 and t == 0),
                                    stop=(kg == NKG_SOC - 1 and t == KG_SOC - 1))
                        if not last:
                            nc.scalar.copy(hsoc_nx[:, m, :], psum_m[:])
                        nc.vector.tensor_add(acc_soc[:, m, :],
                                             acc_soc[:, m, :], psum_m[:])

                def ui_quad(q):
                    for gi, (h, c0, gch, gq) in enumerate(gathers):
                        if gq != q:
                            continue
                        src = src1 if h == 0 else src2
                        msgs = msgp.tile([128, gch, HID2], dt.bfloat16,
                                         tag="msgs")
                        nc.gpsimd.dma_gather(
                            out_ap=msgs[:], in_ap=src[:, :],
                            idxs_ap=idx_sb[:, c0 * 8:(c0 + gch) * 8],
                            num_idxs=gch * 128, num_idxs_reg=gch * 128,
                            elem_size=HID2, queue_num=gi % 2)
                        m_t = ohp.tile([128, gch, WIN], dt.bfloat16, tag="m_t")
                        nc.scalar.dma_start(m_t[:], m_d[:, c0:c0 + gch, :])
                        for j in range(gch):
                            h_, w, first, lastc, _q = meta[c0 + j]
                            pair, sub = w >> 1, w & 1
                            if first and sub == 0:
                                psum_w[pair] = uips.tile(
                                    [128, HID2], dt.float32, tag="uips",
                                    name=f"uips_h{hop}_{h_}_{pair}")
                            ps = psum_w[pair]
                            nc.tensor.matmul(ps[64 * sub:64 * sub + 64, :],
                                             m_t[:, j, :], msgs[:, j, :],
                                             start=first, stop=lastc,
                                             skip_group_check=True)
                            if lastc and sub == 1:
                                if last:
                                    nc.vector.tensor_add(
                                        acc_ui[:, pair, :],
                                        acc_ui[:, pair, :], ps[:])
                                elif h_ == 0:
                                    nc.scalar.copy(hf32[:, pair, :], ps[:])
                                else:
                                    nc.vector.tensor_add(
                                        hf32[:, pair, :], hf32[:, pair, :],
                                        ps[:])

                NP1 = UI1 // 128   # 25 pair-blocks in half 1
                # ---- half 1: ui quad 0, social m 0..4, AG1
                ui_quad(0)
                social_half(0, 5)
                if not last:
                    nc.vector.tensor_add(acc_ui[:, :NP1, :],
                                         acc_ui[:, :NP1, :], hf32[:, :NP1, :])
                    nc.vector.tensor_copy(hui_nx[:, :NP1, :], hf32[:, :NP1, :])
                    b1 = dramp.tile([R1, HID2], dt.bfloat16, tag="b1")
                    nc.sync.dma_start(
                        b1[0:UI1, :].rearrange("(t p) f -> p t f", p=128),
                        hui_nx[:, :NP1, :])
                    nc.sync.dma_start(
                        b1[UI1:R1, :].rearrange("(t p) f -> p t f", p=128),
                        hsoc_nx[:, :5, :])
                    nc.gpsimd.collective_compute(
                        "AllGather", mybir.AluOpType.bypass,
                        ins=[b1.opt()], outs=[nbuf1.opt()],
                        replica_groups=rg)
                # ---- half 2: ui quad 1, social m 5..9, AG2
                ui_quad(1)
                social_half(5, NM_SOC)
                if not last:
                    nc.vector.tensor_add(acc_ui[:, NP1:, :],
                                         acc_ui[:, NP1:, :], hf32[:, NP1:, :])
                    nc.vector.tensor_copy(hui_nx[:, NP1:, :], hf32[:, NP1:, :])
                    b2 = dramp.tile([R2, HID2], dt.bfloat16, tag="b2")
                    nc.sync.dma_start(
                        b2[0:UI2, :].rearrange("(t p) f -> p t f", p=128),
                        hui_nx[:, NP1:, :])
                    nc.sync.dma_start(
                        b2[UI2:R2, :].rearrange("(t p) f -> p t f", p=128),
                        hsoc_nx[:, 5:, :])
                    nc.gpsimd.collective_compute(
                        "AllGather", mybir.AluOpType.bypass,
                        ins=[b2.opt()], outs=[nbuf2.opt()],
                        replica_groups=rg)
                    src1, src2 = nbuf1, nbuf2

            nc.sync.dma_start(out_ui.rearrange("(t p) f -> p t f", p=128),
                              acc_ui[:])
            nc.sync.dma_start(out_soc.rearrange("(t p) f -> p t f", p=128),
                              acc_soc[:])
    nc.compile()
    return nc


# ---------------------------------------------------------------- kernel

def prepare(users, pos, neg, user_embs, item_embs, user1_embs, item1_embs,
            user2_embs, item2_embs, S, A_rows, A_cols, A_vals, hop):
    """Host prep + build: returns (nc, in_maps)."""
    assert int(hop) == HOP

    user_embs = np.asarray(user_embs, np.float32)
    item_embs = np.asarray(item_embs, np.float32)
    user1_embs = np.asarray(user1_embs, np.float32)
    item1_embs = np.asarray(item1_embs, np.float32)
    user2_embs = np.asarray(user2_embs, np.float32)
    item2_embs = np.asarray(item2_embs, np.float32)

    ego = np.hstack([np.vstack([user_embs, item_embs]),
                     np.vstack([user2_embs, item2_embs])])          # [50000,128]
    hsoc = np.hstack([user_embs, user1_embs])                       # [10000,128]
    ego_pad = _pad_table(ego.astype(bf16), CH_UI, PAD_UI)           # [50176,128]
    hsoc_pad = _pad_table(hsoc.astype(bf16), CH_SOC, PAD_SOC)       # [10240,128]
    # split-AG buffer layout: per rank [UI1 ego | SOC_H hsoc] / [UI2 | SOC_H]
    buf_a = np.zeros((T1, HID2), bf16)
    buf_b = np.zeros((T2, HID2), bf16)
    for r in range(N_CORES):
        buf_a[r * R1: r * R1 + UI1] = ego_pad[r * PAD_UI: r * PAD_UI + UI1]
        buf_a[r * R1 + UI1:(r + 1) * R1] = hsoc_pad[r * PAD_SOC: r * PAD_SOC + SOC_H]
        buf_b[r * R2: r * R2 + UI2] = ego_pad[r * PAD_UI + UI1:(r + 1) * PAD_UI]
        buf_b[r * R2 + UI2:(r + 1) * R2] = hsoc_pad[r * PAD_SOC + SOC_H:(r + 1) * PAD_SOC]

    edge_maps, chunks98, NCH = _prep_edges(A_rows, A_cols, A_vals)
    st_all = _prep_social(S)
    iota_np = np.arange(128, dtype=np.float32)

    in_maps = []
    for k in range(N_CORES):
        idx_rep, rows_l, vals_l = edge_maps[k]
        # one-hot scatter matrices, hop-invariant: M[p, c, j] = val * (row == j)
        m_host = ((rows_l.astype(np.float32)[:, :, None] == iota_np[None, None, :])
                  * vals_l.astype(np.float32)[:, :, None]).astype(bf16)
        in_maps.append({
            "ego0_a": buf_a,
            "ego0_b": buf_b,
            "ego_ch": np.ascontiguousarray(ego_pad[k * PAD_UI:(k + 1) * PAD_UI]),
            "hsoc_ch": np.ascontiguousarray(hsoc_pad[k * PAD_SOC:(k + 1) * PAD_SOC]),
            "st": st_all[k],
            "idx": idx_rep,
            "m": m_host,
        })

    nc = _build(chunks98, NCH)
    return nc, in_maps


def finish(results, users, pos, neg, item1_embs):
    """Assemble full outputs from per-core results."""
    inv = 1.0 / (HOP + 1)
    acc_ui = np.concatenate(
        [results[k]["acc_ui"][:CH_UI] for k in range(N_CORES)], 0) * inv
    acc_soc = np.concatenate(
        [results[k]["acc_soc"][:CH_SOC] for k in range(N_CORES)], 0) * inv

    all_user_S = acc_soc[:, :HID]
    all_user_soc1 = acc_soc[:, HID:]
    all_user_A = acc_ui[:N_USERS, :HID]
    all_item = acc_ui[N_USERS:, :HID]
    all_user_rating = acc_ui[:N_USERS, HID:]
    all_item_rating = acc_ui[N_USERS:, HID:]
    all_user = 0.5 * all_user_S + 0.5 * all_user_A

    users = np.asarray(users).astype(np.int64)
    pos = np.asarray(pos).astype(np.int64)
    neg = np.asarray(neg).astype(np.int64)
    item1_embs = np.asarray(item1_embs, np.float32)
    return (all_user[users], all_item[pos], all_item[neg],
            all_user_S, all_user_A,
            all_user_soc1[users], item1_embs[pos], item1_embs[neg],
            all_user_rating[users], all_item_rating[pos],
            all_item_rating[neg])


def kernel(users, pos, neg, user_embs, item_embs, user1_embs, item1_embs,
           user2_embs, item2_embs, S, A_rows, A_cols, A_vals, hop):
    global LAST_EXEC_NS
    nc, in_maps = prepare(users, pos, neg, user_embs, item_embs, user1_embs,
                          item1_embs, user2_embs, item2_embs, S,
                          A_rows, A_cols, A_vals, hop)
    trace = bool(int(os.environ.get("KERNEL_TRACE", "0")))
    res = bass_utils.run_bass_kernel_spmd(
        nc, in_maps, core_ids=list(range(N_CORES)), trace=trace)
    LAST_EXEC_NS = res.exec_time_ns
    return finish(res.results, users, pos, neg, item1_embs)
